# revision 25
# baseline (speedup 1.0000x reference)
"""Causal self-attention on 8 TRN2 NeuronCores.

Sharding: core c handles batch b = c//2 and head-group g = c%2 (8 of 16 heads).
Each core computes its partial y^T = w_proj[slice].T @ o^T (contraction over its
512 o-channels); the host sums the two partials per batch and adds b_proj.

Shapes (hardcoded): B=4, T=2048, C=1024, H=16, HD=64.
"""

import numpy as np

B, T, C, H = 4, 2048, 1024, 16
HD = C // H          # 64
G = 2                # head groups
NHL = H // G         # 8 heads per core
GQ = NHL * HD        # 512 channel slice per core
P = 128
NT = T // P          # 16 token tiles / k-chunks
NCHUNK = C // P      # 8 contraction chunks for qkv
SCALE = 1.0 / float(np.sqrt(HD))

_PROGRAM = None


def _emit(ctx, tc, aps, mybir, bass):
    import contextlib

    nc = tc.nc
    f32 = mybir.dt.float32
    f32r = mybir.dt.float32r
    bf16 = mybir.dt.bfloat16
    EXP = mybir.ActivationFunctionType.Exp

    x_d, wqkv_d, bqk_d, bv_d, wp_d, yT_d = (
        aps["x"], aps["wqkv"], aps["bqk"], aps["bv"], aps["wp"], aps["yT"],
    )

    # ---------------- pools ----------------
    const = ctx.enter_context(tc.tile_pool(name="const", bufs=1))
    dramp = ctx.enter_context(tc.tile_pool(name="dramp", bufs=1, space="DRAM"))
    # psum: main 2x[128,1024] (4 banks) + pv 4x[128,512] (4 banks)
    ps_main = ctx.enter_context(tc.tile_pool(name="ps_main", bufs=2, space="PSUM"))
    ps_pv = ctx.enter_context(tc.tile_pool(name="ps_pv", bufs=4, space="PSUM"))

    qkp = ctx.enter_context(tc.tile_pool(name="qkp", bufs=8))
    vap = ctx.enter_context(tc.tile_pool(name="vap", bufs=16))
    ptp = ctx.enter_context(tc.tile_pool(name="ptp", bufs=3))
    otp = ctx.enter_context(tc.tile_pool(name="otp", bufs=2))
    rcp = ctx.enter_context(tc.tile_pool(name="rcp", bufs=2))

    # constants
    identity = const.tile([P, P], f32)
    from concourse.masks import make_identity
    make_identity(nc, identity)
    bqk_sb = const.tile([P, 8], f32)
    nc.sync.dma_start(bqk_sb[:], bqk_d[:])
    bvb = const.tile([P, GQ], f32)
    nc.sync.dma_start(bvb[:], bv_d[None, :].to_broadcast((P, GQ)))
    ones8 = const.tile([P, NHL, 1], f32)
    nc.vector.memset(ones8[:], 1.0)

    odram = dramp.tile([GQ, T], f32r, space="DRAM")

    # ---------------- phase A: load x, build xT ----------------
    stackAB = contextlib.ExitStack()
    xTp = stackAB.enter_context(tc.tile_pool(name="xTp", bufs=8))
    wqkp = stackAB.enter_context(tc.tile_pool(name="wqkp", bufs=4))
    wvp = stackAB.enter_context(tc.tile_pool(name="wvp", bufs=1))
    stackA = contextlib.ExitStack()
    xp = stackA.enter_context(tc.tile_pool(name="xp", bufs=2))

    xT = []  # 8 tiles [128 c, 2048 t]
    for r in range(NCHUNK):
        t_ = xTp.tile([P, T], f32r, name=f"xT{r}", tag="xT")
        xT.append(t_)

    for tg in range(NT // 2):  # groups of 2 t-tiles
        xts = []
        for tt in range(2):
            t = 2 * tg + tt
            x_t = xp.tile([P, C], f32, name=f"x_{t}", tag="x")
            nc.sync.dma_start(x_t[:], x_d[t * P:(t + 1) * P, :])
            xts.append(x_t)
        for r in range(NCHUNK):
            tp = ps_main.tile([P, 256], f32, name=f"tp_{tg}_{r}", tag="main")
            for tt in range(2):
                nc.tensor.transpose(
                    tp[:, tt * P:(tt + 1) * P],
                    xts[tt][:, r * P:(r + 1) * P],
                    identity,
                )
            nc.vector.tensor_copy(xT[r][:, tg * 256:(tg + 1) * 256], tp[:])
    stackA.close()

    # ---------------- phase B: qkv ----------------
    qkT = []  # bf16 tiles [128 c', 2048 t]; 0..3 = qT, 4..7 = kT
    for ct in range(8):
        o_t = qkp.tile([P, T], bf16, name=f"qkT{ct}", tag="qkT")
        qkT.append(o_t)

    vaug = []  # [128 k, 8 heads, 65] per k-chunk (col 64 = ones for denom)
    for t in range(NT):
        va = vap.tile([P, NHL, HD + 1], f32r, name=f"vaug{t}", tag="vaug")
        nc.vector.tensor_copy(va[:, :, HD:HD + 1], ones8[:])
        vaug.append(va)

    wqkv_r = wqkv_d.rearrange("(a p) n -> p a n", p=P)  # [128, 8, 1536]

    wqk_tiles = {}

    def emit_qk_half(ct, twp):
        # one [128,1024] half of output tile ct (q cols twp*1024..)
        if ct not in wqk_tiles:
            col0 = ct * P
            w_t = wqkp.tile([P, NCHUNK, P], f32r, name=f"wqk_{ct}", tag="wqk")
            nc.sync.dma_start(w_t[:], wqkv_r[:, :, col0:col0 + P])
            wqk_tiles[ct] = w_t
        w_t = wqk_tiles[ct]
        for sw in range(2):
            ps = ps_pv.tile([P, 512], f32, name=f"qkps_{ct}_{twp}_{sw}",
                            tag="ps_pv")
            for a in range(NCHUNK):
                nc.tensor.matmul(
                    ps[:],
                    w_t[:, a, :],
                    xT[a][:, twp * 1024 + sw * 512:twp * 1024 + (sw + 1) * 512],
                    start=(a == 0),
                    stop=(a == NCHUNK - 1),
                )
            nc.vector.tensor_scalar_add(
                qkT[ct][:, twp * 1024 + sw * 512:twp * 1024 + (sw + 1) * 512],
                ps[:], bqk_sb[:, ct:ct + 1]
            )

    wv_tile = {}

    def emit_v(t0, t1):
        if not wv_tile:
            w_t = wvp.tile([P, NCHUNK, GQ], f32r, name="wv", tag="wv")
            nc.sync.dma_start(w_t[:], wqkv_r[:, :, 2 * GQ:3 * GQ])
            wv_tile[0] = w_t
        w_t = wv_tile[0]
        for t in range(t0, t1):
            ps = ps_pv.tile([P, GQ], f32, name=f"vps_{t}", tag="ps_pv")
            for a in range(NCHUNK):
                nc.tensor.matmul(
                    ps[:],
                    xT[a][:, t * P:(t + 1) * P],
                    w_t[:, a, :],
                    start=(a == 0),
                    stop=(a == NCHUNK - 1),
                )
            nc.vector.tensor_add(
                vaug[t][:, :, 0:HD],
                ps[:].rearrange("p (h d) -> p h d", h=NHL),
                bvb[:].rearrange("p (h d) -> p h d", h=NHL),
            )

    # ---------------- phase C: attention ----------------
    # Head pairs: head A on PE row strip 0, head B on strip 64; score pieces
    # for the two heads live in the two banks of one [128,1024] psum tile, so
    # the row-packed matmuls run concurrently and one exp covers both heads.
    # Quarter-outer loop: each 512-wide q-window accumulates PV fully, then
    # normalizes while the next window runs (pv pool rotation hides it).
    def attn_pair(hp):
        qt = qkT[hp]
        kt = qkT[4 + hp]
        for m in range(4):  # quarter windows of 512 q
            ws = m * 512
            pvt = {}
            for hh in range(2):
                pvt[hh] = ps_pv.tile(
                    [P, 512], f32, name=f"pv_{hp}_{m}_{hh}", tag="ps_pv")
            for i in range(4 * m + 4):  # causal k-chunks for this window
                s = max(i * P, ws)
                o = s - ws
                # head A piece in cols [o, 512), head B in [512, 1024-o)
                sc = ps_main.tile([P, 1024], f32, name=f"sc_{hp}_{m}_{i}",
                                  tag="main")
                for hh in range(2):
                    r0 = hh * HD
                    c0 = o if hh == 0 else 512
                    nc.tensor.matmul(
                        sc[:, c0:c0 + 512 - o],
                        kt[r0:r0 + HD, i * P:(i + 1) * P],
                        qt[r0:r0 + HD, s:ws + 512],
                        start=True,
                        stop=True,
                    )
                pt = ptp.tile([P, 1024], f32r, name=f"pt_{hp}_{m}_{i}",
                              tag="pt")
                nc.scalar.activation(pt[:, o:1024 - o], sc[:, o:1024 - o],
                                     EXP, scale=SCALE)
                diag = i * P >= ws
                for hh in range(2):
                    c0 = o if hh == 0 else 512
                    if diag:
                        nc.gpsimd.affine_select(
                            out=pt[:, c0:c0 + P],
                            in_=pt[:, c0:c0 + P],
                            compare_op=mybir.AluOpType.is_ge,
                            fill=0.0,
                            base=0,
                            pattern=[[1, P]],
                            channel_multiplier=-1,
                        )
                    nc.tensor.matmul(
                        pvt[hh][0:HD + 1, o:],
                        vaug[i][:, 2 * hp + hh, :],
                        pt[:, c0:c0 + 512 - o],
                        start=(i == 0),
                        stop=(i == 4 * m + 3),
                    )
            # normalize both heads: ot rows 0:64 = head A, 64:128 = head B
            ot = otp.tile([P, 512], f32r, name=f"ot_{hp}_{m}",
                          tag=("ot3" if hp == 3 else "ot"),
                          bufs=(4 if hp == 3 else None))
            for hh in range(2):
                rc = rcp.tile([1, 512], f32, name=f"rc_{hp}_{m}_{hh}", tag="rc")
                nc.vector.reciprocal(rc[:], pvt[hh][HD:HD + 1, :])
                rcb = rcp.tile([HD, 512], f32, name=f"rcb_{hp}_{m}_{hh}",
                               tag="rcb")
                nc.gpsimd.partition_broadcast(rcb[:], rc[:])
                nc.vector.tensor_mul(
                    ot[hh * HD:(hh + 1) * HD, :], pvt[hh][0:HD, :], rcb[:])
            if hp == 3:
                ot3.append(ot)  # stays in SBUF, feeds proj directly
            else:
                nc.sync.dma_start(odram[hp * P:(hp + 1) * P, ws:ws + 512], ot[:])

    # interleave qkv production with attention so the PE stream stays dense
    # through the ACT-bound attention stretches (keeps HAM warm); emit only
    # what pair 0 quarter 0 needs before starting it.
    ot3 = []
    emit_v(0, 4)
    emit_qk_half(0, 0)
    emit_qk_half(4, 0)
    emit_v(4, 16)
    emit_qk_half(0, 1)
    emit_qk_half(4, 1)
    attn_pair(0)
    for j in range(1, 4):
        for twp in range(2):
            emit_qk_half(j, twp)
            emit_qk_half(4 + j, twp)
        attn_pair(j)

    stackAB.close()  # release x / w / xT pools

    # ---------------- phase D: proj ----------------
    stackD = contextlib.ExitStack()
    orp = stackD.enter_context(tc.tile_pool(name="orp", bufs=3))
    wpp = stackD.enter_context(tc.tile_pool(name="wpp", bufs=1))
    ysp = stackD.enter_context(tc.tile_pool(name="ysp", bufs=3))

    wp_t = wpp.tile([P, 4, C], f32r, name="wp", tag="wp")
    nc.sync.dma_start(wp_t[:], wp_d.rearrange("(a p) n -> p a n", p=P))
    oTr = []
    for a in range(3):
        o_t = orp.tile([P, T], f32r, name=f"oTr{a}", tag="oTr")
        nc.sync.dma_start(o_t[:], odram[a * P:(a + 1) * P, :])
        oTr.append(o_t)
    for mt in range(NCHUNK):  # cout tiles
        for twp in range(2):
            ps = ps_main.tile([P, 1024], f32, name=f"yps_{mt}_{twp}", tag="main")
            for a in range(4):
                for sw in range(2):
                    rhs = (oTr[a][:, twp * 1024 + sw * 512:twp * 1024 + (sw + 1) * 512]
                           if a < 3 else ot3[2 * twp + sw][:, :])
                    nc.tensor.matmul(
                        ps[:, sw * 512:(sw + 1) * 512],
                        wp_t[:, a, mt * P:(mt + 1) * P],
                        rhs,
                        start=(a == 0),
                        stop=(a == 3),
                    )
            ys = ysp.tile([P, 1024], f32, name=f"ys_{mt}_{twp}", tag="ys")
            nc.scalar.copy(ys[:], ps[:])
            nc.sync.dma_start(
                yT_d[mt * P:(mt + 1) * P, twp * 1024:(twp + 1) * 1024], ys[:]
            )
    stackD.close()


def _build_program():
    import contextlib

    import concourse.bass as bass
    import concourse.mybir as mybir
    import concourse.tile as tile
    from concourse import bacc

    nc = bacc.Bacc("TRN2", target_bir_lowering=False, debug=False, num_devices=8)
    f32 = mybir.dt.float32
    aps = {
        "x": nc.dram_tensor("x", [T, C], f32, kind="ExternalInput").ap(),
        "wqkv": nc.dram_tensor("wqkv", [C, 3 * GQ], mybir.dt.float32r, kind="ExternalInput").ap(),
        "bqk": nc.dram_tensor("bqk", [P, 8], f32, kind="ExternalInput").ap(),
        "bv": nc.dram_tensor("bv", [GQ], f32, kind="ExternalInput").ap(),
        "wp": nc.dram_tensor("wp", [GQ, C], mybir.dt.float32r, kind="ExternalInput").ap(),
        "yT": nc.dram_tensor("yT", [C, T], f32, kind="ExternalOutput").ap(),
    }
    with tile.TileContext(nc) as tc:
        with contextlib.ExitStack() as ctx:
            _emit(ctx, tc, aps, mybir, bass)
    nc.compile()
    return nc


def get_program():
    global _PROGRAM
    if _PROGRAM is None:
        _PROGRAM = _build_program()
    return _PROGRAM


def make_in_maps(x, w_qkv, b_qkv, w_proj):
    x = np.asarray(x, np.float32)
    w_qkv = np.asarray(w_qkv, np.float32)
    b_qkv = np.asarray(b_qkv, np.float32)
    w_proj = np.asarray(w_proj, np.float32)
    in_maps = []
    for c in range(8):
        b = c // 2
        g = c % 2
        q0 = g * GQ
        wq = w_qkv[:, q0:q0 + GQ]
        wk = w_qkv[:, C + q0:C + q0 + GQ]
        wv = w_qkv[:, 2 * C + q0:2 * C + q0 + GQ]
        wqkv = np.ascontiguousarray(np.concatenate([wq, wk, wv], axis=1))
        bq = b_qkv[q0:q0 + GQ]
        bk = b_qkv[C + q0:C + q0 + GQ]
        bqk = np.ascontiguousarray(np.concatenate([bq, bk]).reshape(8, P).T)
        bv = np.ascontiguousarray(b_qkv[2 * C + q0:2 * C + q0 + GQ])
        in_maps.append({
            "x": np.ascontiguousarray(x[b]),
            "wqkv": wqkv,
            "bqk": bqk,
            "bv": bv,
            "wp": np.ascontiguousarray(w_proj[q0:q0 + GQ, :]),
        })
    return in_maps


def combine_outputs(outs, b_proj):
    b_proj = np.asarray(b_proj, np.float32)
    y = np.empty((B, T, C), np.float32)
    for b in range(B):
        acc = outs[2 * b] + outs[2 * b + 1]  # [C, T]
        y[b] = acc.T + b_proj
    return y


def kernel(x, w_qkv, b_qkv, w_proj, b_proj, _trace=False):
    from concourse import bass_utils

    nc = get_program()
    in_maps = make_in_maps(x, w_qkv, b_qkv, w_proj)
    res = bass_utils.run_bass_kernel_spmd(
        nc, in_maps, core_ids=list(range(8)), trace=_trace
    )
    outs = [r["yT"] for r in res.results]
    y = combine_outputs(outs, b_proj)
    if _trace:
        return y, res
    return y
